# revision 4
# baseline (speedup 1.0000x reference)
"""Single-head causal attention (B=8, T=2048, E=H=1024) on 8 TRN2 NeuronCores.

Strategy: data-parallel over batch (one batch element per core). Per core:
  warmup:   3 fp32 matmuls on a scratch tile keep the PE busy during the
            initial input DMA so the HAM clock-gate is already at 2.4 GHz
            when real work starts (cold PE runs at 1.2 GHz).
  phase A2: v = x@Wv.T [T,H] runs FIRST (x.T blocks stationary, Wv.T
            moving), resident in SBUF as bf16. Its first ~14us are
            DMA-paced (true of any first phase); meanwhile the whole A1
            working set (x.T + Wq/Wk) prefetches on the other two DMA
            rings.
  phase A1: qT = (x@Wq.T).T and kT = (x@Wk.T).T  [H,T], contraction over
            E, hm outer / t4 inner (x.T resident, weights stream through
            2 rotating buffers). qT and kT are cast to bf16 and BOTH stay
            resident in SBUF — no DRAM spill.
  phase B:  causal flash attention over t-chunks of 256 in the S^T
            orientation: S^T[s,t] = sum_h kT[h,s]*qT[h,t] (bf16 operands,
            fp32 PSUM); softmax weights come out as wT[s_block, t] tiles
            feeding O[t,h] += wT.T @ v[s_block] directly. Row sums ride
            along as matmuls against a ones column. Diagonal masking uses
            one precomputed [128,128] bf16 triangle mask applied with a
            DVE multiply; the fully-masked t-half of the last s-block of
            each chunk is skipped (scores computed at N=128).

DMA plan: a single HWDGE ring is trigger-rate-limited (~0.7us/trigger)
and any semaphore-gated trigger blocks everything queued behind it on
that ring, so the load is split across all three rings (sync Q1,
scalar-HWDGE Q10, gpsimd-SWDGE Q0) in per-ring consumption order:
  gpsimd: ramp tail + W0/W1 + x chunks 0-1  (fires early, no gating)
  scalar: ramp mid + x chunks 2-3           (before its vt/qt copies)
  sync:   ramp head + xV stream (prefetch-depth-3 triggers) + W2..W7
          in-loop (gated, but nothing critical queued behind)

Projections run in float32r (full-rate ~12-bit-significand fp32); the
attention part runs in bf16 (~4e-3 end-to-end vs the fp32 reference,
correctness gate is 2e-2).
"""

import numpy as np

import concourse.bacc as bacc
import concourse.mybir as mybir
import concourse.tile as tile
from concourse.bass_utils import run_bass_kernel_spmd

B, T, E, H = 8, 2048, 1024, 1024
N_CORES = 8
SCALE = float(E) ** -0.5

DT = mybir.dt.float32r
BF = mybir.dt.bfloat16
F32 = mybir.dt.float32

TCB = 256            # phase-B t-chunk width
N_TCB = T // TCB     # 8
N_EB = E // 128      # 8  e-blocks
N_HB = H // 128      # 8  h-blocks
N_SB = T // 128      # 16 s-blocks


def build_program():
    nc = bacc.Bacc("TRN2", target_bir_lowering=False, debug=False,
                   num_devices=N_CORES)

    # host-prepped layouts: every DMA reads long contiguous runs
    xT_d = nc.declare_dram_parameter("xA", [4, 128, N_EB, 512], DT,
                                     isOutput=False)   # [t4][p][ek][t]
    xV_d = nc.declare_dram_parameter("xV", [8, 128, N_EB, 256], DT,
                                     isOutput=False)   # [t8][p][ek][t]
    wqT_d = nc.declare_dram_parameter("WqT", [N_HB, 128, N_EB, 128], DT,
                                      isOutput=False)  # [hm][p][ek][h]
    wkT_d = nc.declare_dram_parameter("WkT", [N_HB, 128, N_EB, 128], DT,
                                      isOutput=False)
    wvT_d = nc.declare_dram_parameter("WvT", [E, H], DT, isOutput=False)
    out_d = nc.declare_dram_parameter("out", [T, H], F32, isOutput=True)

    with tile.TileContext(nc) as tc:
        with (
            tc.tile_pool(name="misc", bufs=1) as pool_misc,
            tc.tile_pool(name="v", bufs=1) as pool_v,
        ):
            vt = [pool_v.tile([128, H], BF, tag=f"v{j}", name=f"v{j}")
                  for j in range(N_SB)]

            ones_f = pool_misc.tile([128, 1], F32, tag="ones_f", name="ones_f")
            ones_b = pool_misc.tile([128, 1], BF, tag="ones_b", name="ones_b")
            mask = pool_misc.tile([128, 128], BF, tag="mask", name="mask")
            dummy = pool_misc.tile([128, 512], F32, tag="dummy", name="dummy")
            nc.gpsimd.memset(ones_f[:], 1.0)
            nc.gpsimd.memset(dummy[:], 0.0)
            nc.gpsimd.memset(mask[:], 1.0)
            nc.vector.tensor_copy(ones_b[:], ones_f[:])
            # triangle mask: keep s <= t within a 128x128 block
            nc.gpsimd.affine_select(
                out=mask[:], in_=mask[:],
                compare_op=mybir.AluOpType.is_ge,
                fill=0.0, base=0, channel_multiplier=-1,
                pattern=[[1, 128]])

            with (
                tc.tile_pool(name="xf", bufs=1) as pool_xf,
                tc.tile_pool(name="wqk", bufs=2) as pool_wqk,
            ):
                xft = [pool_xf.tile([128, N_EB, 512], DT, tag=f"xf{t4}",
                                    name=f"xf{t4}") for t4 in range(4)]
                wqt = {}
                wkt = {}
                for hm in range(N_HB):
                    wqt[hm] = pool_wqk.tile([128, N_EB, 128], DT, tag="wqb",
                                            name=f"wqb{hm}")
                    wkt[hm] = pool_wqk.tile([128, N_EB, 128], DT, tag="wkb",
                                            name=f"wkb{hm}")

                # ------------- phase A2: v (resident, bf16) ----------------
                with (
                    tc.tile_pool(name="wv", bufs=1) as pool_wv,
                    tc.tile_pool(name="xv0", bufs=1) as pool_xv0,
                    tc.tile_pool(name="x2", bufs=4) as pool_x2,
                    tc.tile_pool(name="pv", bufs=4, space="PSUM") as psum_v,
                    tc.tile_pool(name="pd", bufs=1, space="PSUM") as psum_d,
                ):
                    # PE warmup: 3 fp32 matmuls (~1us each) on scratch. They
                    # depend only on the gpsimd memsets, so they run during
                    # the initial DMA and un-throttle the HAM clock gate.
                    dummy_ps = psum_d.tile([1, 512], F32, tag="dummy_ps",
                                           name="dummy_ps")
                    for i in range(3):
                        nc.tensor.matmul(dummy_ps[:], ones_f[:], dummy[:],
                                         start=True, stop=True)

                    wvh = [[pool_wv.tile([128, 512], DT, tag=f"wv{k}_{hc}",
                                         name=f"wv{k}_{hc}")
                            for hc in range(2)] for k in range(N_EB)]
                    xv0 = [pool_xv0.tile([128, 2, 256], DT, tag=f"xv0_{i}",
                                         name=f"xv0_{i}") for i in range(4)]
                    xvt = {t8: pool_x2.tile([128, N_EB, 256], DT, tag="xv",
                                            name=f"xv_{t8}")
                           for t8 in range(1, 8)}

                    def wv_dma(eng, k, hc):
                        eng.dma_start(
                            wvh[k][hc][:],
                            wvT_d[k * 128:(k + 1) * 128,
                                  hc * 512:(hc + 1) * 512])

                    # ---- sync ring: ramp head, then the xV stream --------
                    wv_dma(nc.sync, 0, 0)
                    nc.sync.dma_start(xv0[0][:], xV_d[0, :, 0:2, :])
                    wv_dma(nc.sync, 1, 0)
                    nc.sync.dma_start(xv0[1][:], xV_d[0, :, 2:4, :])
                    wv_dma(nc.sync, 2, 0)
                    nc.sync.dma_start(xv0[2][:], xV_d[0, :, 4:6, :])
                    wv_dma(nc.sync, 3, 0)
                    nc.sync.dma_start(xv0[3][:], xV_d[0, :, 6:8, :])
                    for t8 in range(1, 4):
                        nc.sync.dma_start(xvt[t8][:], xV_d[t8, :, :, :])

                    # ---- scalar ring: ramp mid + x chunks 2-3 ------------
                    for k in range(4, N_EB):
                        wv_dma(nc.scalar, k, 0)
                    for k in range(0, 4):
                        wv_dma(nc.scalar, k, 1)
                    nc.scalar.dma_start(xft[2][:], xT_d[2, :, :, :])
                    nc.scalar.dma_start(xft[3][:], xT_d[3, :, :, :])

                    # ---- gpsimd ring: ramp tail + W0/W1 + x chunks 0-1 ---
                    for k in range(4, N_EB):
                        wv_dma(nc.gpsimd, k, 1)
                    nc.gpsimd.dma_start(wqt[0][:], wqT_d[0, :, :, :])
                    nc.gpsimd.dma_start(xft[0][:], xT_d[0, :, :, :])
                    nc.gpsimd.dma_start(wkt[0][:], wkT_d[0, :, :, :])
                    nc.gpsimd.dma_start(wqt[1][:], wqT_d[1, :, :, :])
                    nc.gpsimd.dma_start(wkt[1][:], wkT_d[1, :, :, :])
                    nc.gpsimd.dma_start(xft[1][:], xT_d[1, :, :, :])

                    def xv_slice(t8, ek, sl):
                        if t8 == 0:
                            return xv0[ek // 2][:, ek % 2, sl]
                        return xvt[t8][:, ek, sl]

                    with nc.named_scope("proj_v"):
                        for t8 in range(T // 256):
                            if 1 <= t8 and t8 + 3 < 8:
                                # prefetch-depth-3 trigger: waits only on
                                # the buffer freed by leg t8-1
                                nc.sync.dma_start(xvt[t8 + 3][:],
                                                  xV_d[t8 + 3, :, :, :])
                            for ss in range(2):
                                j = t8 * 2 + ss
                                sl = slice(ss * 128, (ss + 1) * 128)
                                for hc in range(2):
                                    pv = psum_v.tile([128, 512], F32,
                                                     tag="pv",
                                                     name=f"pv_{t8}_{ss}_{hc}")
                                    for ek in range(N_EB):
                                        nc.tensor.matmul(
                                            pv[:], xv_slice(t8, ek, sl),
                                            wvh[ek][hc][:],
                                            start=(ek == 0),
                                            stop=(ek == N_EB - 1))
                                    if hc == 0:
                                        nc.vector.tensor_copy(
                                            vt[j][:, 0:512], pv[:])
                                    else:
                                        nc.scalar.copy(
                                            vt[j][:, 512:1024], pv[:])

                # ------------- phase A1: qT + kT (both resident) -----------
                with (
                    tc.tile_pool(name="kt", bufs=1) as pool_kt,
                    tc.tile_pool(name="qt", bufs=1) as pool_qt,
                ):
                    kt = [pool_kt.tile([128, T], BF, tag=f"kt{k}",
                                       name=f"kt{k}") for k in range(N_HB)]
                    qt = [pool_qt.tile([128, T], BF, tag=f"qt{k}",
                                       name=f"qt{k}") for k in range(N_HB)]

                    with tc.tile_pool(name="pa", bufs=3,
                                      space="PSUM") as psum_a:
                        with nc.named_scope("proj_qk"):
                            for hm in range(N_HB):
                                if hm + 2 < N_HB:
                                    # W stream rides the (now idle) sync
                                    # ring; triggers are gated by buffer
                                    # rotation but nothing critical queues
                                    # behind them
                                    nc.sync.dma_start(wqt[hm + 2][:],
                                                      wqT_d[hm + 2, :, :, :])
                                    nc.sync.dma_start(wkt[hm + 2][:],
                                                      wkT_d[hm + 2, :, :, :])
                                for t4 in range(4):
                                    pq = psum_a.tile([128, 512], F32,
                                                     tag="pq",
                                                     name=f"pq_{hm}_{t4}")
                                    pk = psum_a.tile([128, 512], F32,
                                                     tag="pk",
                                                     name=f"pk_{hm}_{t4}")
                                    for ek in range(N_EB):
                                        nc.tensor.matmul(
                                            pq[:], wqt[hm][:, ek, :],
                                            xft[t4][:, ek, :],
                                            start=(ek == 0),
                                            stop=(ek == N_EB - 1))
                                    for ek in range(N_EB):
                                        nc.tensor.matmul(
                                            pk[:], wkt[hm][:, ek, :],
                                            xft[t4][:, ek, :],
                                            start=(ek == 0),
                                            stop=(ek == N_EB - 1))
                                    nc.scalar.copy(
                                        qt[hm][:, t4 * 512:(t4 + 1) * 512],
                                        pq[:])
                                    nc.vector.tensor_copy(
                                        kt[hm][:, t4 * 512:(t4 + 1) * 512],
                                        pk[:])

                    # ------------- phase B: causal attention ---------------
                    with (
                        tc.tile_pool(name="wt", bufs=4) as pool_wt,
                        tc.tile_pool(name="ob", bufs=6) as pool_ob,
                        tc.tile_pool(name="sm", bufs=4) as pool_sm,
                        tc.tile_pool(name="pb", bufs=1, space="PSUM") as psum_b,
                    ):
                        with nc.named_scope("attn"):
                            for c in range(N_TCB):
                                n_j = 2 * c + 2
                                o_ps = [psum_b.tile([128, 512], F32,
                                                    tag=f"O{i}",
                                                    name=f"O_{c}_{i}")
                                        for i in range(4)]
                                rs_ps = psum_b.tile([1, TCB], F32, tag="rs",
                                                    name=f"rs_{c}")

                                def scores(j, c=c, n_j=n_j):
                                    # last s-block: t-half 0 fully masked ->
                                    # compute only the 128 t-half-1 columns
                                    half = (j == n_j - 1)
                                    off = 128 if half else 0
                                    s_ps = psum_b.tile([128, TCB], F32,
                                                       tag=f"S{j % 2}",
                                                       name=f"S_{c}_{j}")
                                    for hk in range(N_HB):
                                        nc.tensor.matmul(
                                            s_ps[:, off:TCB],
                                            kt[hk][:, j * 128:(j + 1) * 128],
                                            qt[hk][:, c * TCB + off:
                                                   (c + 1) * TCB],
                                            start=(hk == 0),
                                            stop=(hk == N_HB - 1))
                                    wt = pool_wt.tile([128, TCB], BF,
                                                      tag="wt",
                                                      name=f"wt_{c}_{j}")
                                    nc.scalar.activation(
                                        wt[:, off:TCB], s_ps[:, off:TCB],
                                        mybir.ActivationFunctionType.Exp,
                                        scale=SCALE)
                                    if j == 2 * c:
                                        # diagonal block: t-half 0 triangular
                                        nc.vector.tensor_mul(
                                            wt[:, 0:128], wt[:, 0:128],
                                            mask[:])
                                    elif half:
                                        # block j=2c+1: t-half 1 triangular
                                        nc.vector.tensor_mul(
                                            wt[:, 128:TCB], wt[:, 128:TCB],
                                            mask[:])
                                    return wt

                                def o_accum(j, wt, c=c, n_j=n_j, o_ps=o_ps,
                                            rs_ps=rs_ps):
                                    first, last = (j == 0), (j == n_j - 1)
                                    off = 128 if last else 0
                                    nc.tensor.matmul(
                                        rs_ps[0:1, off:TCB], ones_b[:],
                                        wt[:, off:TCB],
                                        start=first, stop=last,
                                        skip_group_check=True)
                                    for ts in range(2):
                                        if ts == 0 and last:
                                            # fully masked: zero contribution
                                            continue
                                        wslice = wt[:, ts * 128:(ts + 1) * 128]
                                        last_ts = (j == n_j - 2) if ts == 0 \
                                            else last
                                        for hc in range(2):
                                            nc.tensor.matmul(
                                                o_ps[ts * 2 + hc][:], wslice,
                                                vt[j][:, hc * 512:
                                                      (hc + 1) * 512],
                                                start=first, stop=last_ts)

                                # software pipeline: scores(j+1) ahead of
                                # O(j) so the PE never waits on the exp chain
                                wt_cur = scores(0)
                                for j in range(n_j):
                                    wt_next = scores(j + 1) \
                                        if j + 1 < n_j else None
                                    o_accum(j, wt_cur)
                                    wt_cur = wt_next
                                rs_sb = pool_sm.tile([1, TCB], F32,
                                                     tag="rs_sb",
                                                     name=f"rs_sb_{c}")
                                nc.vector.tensor_copy(rs_sb[:], rs_ps[:])
                                for ts in range(2):
                                    # transpose [1,128]->[128,1] via matmul
                                    rs_col = psum_b.tile([128, 1], F32,
                                                         tag="rs_col",
                                                         name=f"rs_col_{c}_{ts}")
                                    nc.tensor.matmul(
                                        rs_col[:],
                                        rs_sb[0:1, ts * 128:(ts + 1) * 128],
                                        ones_f[0:1, 0:1],
                                        start=True, stop=True)
                                    rec = pool_sm.tile([128, 1], F32,
                                                       tag="rec",
                                                       name=f"rec_{c}_{ts}")
                                    nc.vector.reciprocal(rec[:], rs_col[:])
                                    for hc in range(2):
                                        ob = pool_ob.tile([128, 512], F32,
                                                          tag="ob",
                                                          name=f"ob_{c}_{ts}_{hc}")
                                        if hc == 0:
                                            nc.vector.tensor_scalar_mul(
                                                ob[:], o_ps[ts * 2 + hc][:],
                                                rec[:])
                                        else:
                                            nc.scalar.activation(
                                                ob[:], o_ps[ts * 2 + hc][:],
                                                mybir.ActivationFunctionType.Copy,
                                                scale=rec[:])
                                        out_ap = out_d[
                                            c * TCB + ts * 128:
                                            c * TCB + (ts + 1) * 128,
                                            hc * 512:(hc + 1) * 512]
                                        if c == N_TCB - 1:
                                            nc.sync.dma_start(out_ap, ob[:])
                                        else:
                                            nc.gpsimd.dma_start(out_ap, ob[:])

    nc.compile()
    return nc


_NC_CACHE = None


def _get_program():
    global _NC_CACHE
    if _NC_CACHE is None:
        _NC_CACHE = build_program()
    return _NC_CACHE


def make_in_maps(x, Wk, Wq, Wv):
    x = np.asarray(x, np.float32)
    xT = np.transpose(x, (0, 2, 1))                        # [B, E, T]
    # A1 layout [t4][p][ek][512]: xT[e, t] with e = ek*128 + p
    xA = np.ascontiguousarray(
        xT.reshape(B, N_EB, 128, 4, 512).transpose(0, 3, 2, 1, 4))
    # A2 layout [t8][p][ek][256]
    xV = np.ascontiguousarray(
        xT.reshape(B, N_EB, 128, 8, 256).transpose(0, 3, 2, 1, 4))

    def prep_w(W):   # [H,E] -> W.T [E,H] -> [hm][p][ek][128]
        WT = np.asarray(W, np.float32).T
        return np.ascontiguousarray(
            WT.reshape(N_EB, 128, N_HB, 128).transpose(2, 1, 0, 3))

    WqT = prep_w(Wq)
    WkT = prep_w(Wk)
    WvT = np.ascontiguousarray(np.asarray(Wv, np.float32).T)  # [E, H]
    return [{"xA": xA[b], "xV": xV[b], "WqT": WqT, "WkT": WkT, "WvT": WvT}
            for b in range(B)]


def kernel(x, Wk, Wq, Wv, _trace=False, _tmpdir=None):
    nc = _get_program()
    in_maps = make_in_maps(x, Wk, Wq, Wv)
    res = run_bass_kernel_spmd(nc, in_maps, list(range(N_CORES)),
                               trace=_trace, tmpdir=_tmpdir)
    out = np.stack([res.results[b]["out"] for b in range(B)])
    if _trace:
        kernel.last_result = res
    return out


# revision 5
# speedup vs baseline: 1.0082x; 1.0082x over previous
"""Single-head causal attention (B=8, T=2048, E=H=1024) on 8 TRN2 NeuronCores.

Strategy: data-parallel over batch (one batch element per core). Per core:
  warmup:   3 fp32 matmuls on a scratch tile keep the PE busy during the
            initial input DMA so the HAM clock-gate is already at 2.4 GHz
            when real work starts (cold PE runs at 1.2 GHz).
  phase A2: v = x@Wv.T [T,H] runs FIRST (x.T blocks stationary, Wv.T
            moving), resident in SBUF as bf16. x.T is read from the SAME
            resident tiles phase A1 uses — x is loaded from HBM exactly
            once (20MB total input DMA). The first leg runs ek-outer
            across 4 concurrent PSUM groups so the DMA-paced ramp makes
            sub-microsecond stalls instead of HAM-rethrottling gaps.
  phase A1: qT = (x@Wq.T).T and kT = (x@Wk.T).T  [H,T], contraction over
            E, hm outer / t4 inner (x.T resident, weights stream through
            2 rotating buffers). qT and kT are cast to bf16 and BOTH stay
            resident in SBUF — no DRAM spill.
  phase B:  causal flash attention over t-chunks of 256 in the S^T
            orientation: S^T[s,t] = sum_h kT[h,s]*qT[h,t] (bf16 operands,
            fp32 PSUM); softmax weights come out as wT[s_block, t] tiles
            feeding O[t,h] += wT.T @ v[s_block] directly. Row sums ride
            along as matmuls against a ones column. Diagonal masking uses
            one precomputed [128,128] bf16 triangle mask applied with a
            DVE multiply; the fully-masked t-half of the last s-block of
            each chunk is skipped (scores computed at N=128). The
            scores/accumulate software pipeline runs ACROSS chunk
            boundaries so the final diagonal exp chain and the epilogue
            overlap the next chunk's score matmuls.

DMA plan: a single HWDGE ring is trigger-rate-limited and any
semaphore-gated trigger blocks everything queued behind it, so the 20MB
load is split across all three rings (sync Q1, scalar-HWDGE Q10,
gpsimd-SWDGE Q0) in per-ring consumption order, first-leg tiles
interleaved across rings, W2..W7 gated-streamed on the idle sync ring
during A1.

Projections run in float32r (full-rate ~12-bit-significand fp32); the
attention part runs in bf16 (~4e-3 end-to-end vs the fp32 reference,
correctness gate is 2e-2).
"""

import numpy as np

import concourse.bacc as bacc
import concourse.mybir as mybir
import concourse.tile as tile
from concourse.bass_utils import run_bass_kernel_spmd

B, T, E, H = 8, 2048, 1024, 1024
N_CORES = 8
SCALE = float(E) ** -0.5

DT = mybir.dt.float32r
BF = mybir.dt.bfloat16
F32 = mybir.dt.float32

TCB = 256            # phase-B t-chunk width
N_TCB = T // TCB     # 8
N_EB = E // 128      # 8  e-blocks
N_HB = H // 128      # 8  h-blocks
N_SB = T // 128      # 16 s-blocks


def build_program():
    nc = bacc.Bacc("TRN2", target_bir_lowering=False, debug=False,
                   num_devices=N_CORES)

    # host-prepped layouts: every DMA reads long contiguous runs
    xT_d = nc.declare_dram_parameter("xA", [4, 128, N_EB, 512], DT,
                                     isOutput=False)   # [t4][p][ek][t]
    wqT_d = nc.declare_dram_parameter("WqT", [N_HB, 128, N_EB, 128], DT,
                                      isOutput=False)  # [hm][p][ek][h]
    wkT_d = nc.declare_dram_parameter("WkT", [N_HB, 128, N_EB, 128], DT,
                                      isOutput=False)
    wvT_d = nc.declare_dram_parameter("WvT", [E, H], DT, isOutput=False)
    out_d = nc.declare_dram_parameter("out", [T, H], F32, isOutput=True)

    with tile.TileContext(nc) as tc:
        with (
            tc.tile_pool(name="misc", bufs=1) as pool_misc,
            tc.tile_pool(name="v", bufs=1) as pool_v,
        ):
            vt = [pool_v.tile([128, H], BF, tag=f"v{j}", name=f"v{j}")
                  for j in range(N_SB)]

            ones_f = pool_misc.tile([128, 1], F32, tag="ones_f", name="ones_f")
            ones_b = pool_misc.tile([128, 1], BF, tag="ones_b", name="ones_b")
            mask = pool_misc.tile([128, 128], BF, tag="mask", name="mask")
            dummy = pool_misc.tile([128, 512], F32, tag="dummy", name="dummy")
            nc.gpsimd.memset(ones_f[:], 1.0)
            nc.gpsimd.memset(dummy[:], 0.0)
            nc.gpsimd.memset(mask[:], 1.0)
            nc.vector.tensor_copy(ones_b[:], ones_f[:])
            # triangle mask: keep s <= t within a 128x128 block
            nc.gpsimd.affine_select(
                out=mask[:], in_=mask[:],
                compare_op=mybir.AluOpType.is_ge,
                fill=0.0, base=0, channel_multiplier=-1,
                pattern=[[1, 128]])

            with (
                tc.tile_pool(name="xf", bufs=1) as pool_xf,
                tc.tile_pool(name="wqk", bufs=2) as pool_wqk,
            ):
                # x chunks 0 and 1 are split so the ramp waits on small
                # pieces; chunks 2-3 land long before use
                xf0 = [pool_xf.tile([128, 2, 512], DT, tag=f"xf0_{i}",
                                    name=f"xf0_{i}") for i in range(4)]
                xf1 = [pool_xf.tile([128, 4, 512], DT, tag=f"xf1_{i}",
                                    name=f"xf1_{i}") for i in range(2)]
                xf23 = {t4: pool_xf.tile([128, N_EB, 512], DT, tag=f"xf{t4}",
                                         name=f"xf{t4}") for t4 in (2, 3)}

                def xf_slice(t4, ek, sl=slice(0, 512)):
                    if t4 == 0:
                        return xf0[ek // 2][:, ek % 2, sl]
                    if t4 == 1:
                        return xf1[ek // 4][:, ek % 4, sl]
                    return xf23[t4][:, ek, sl]

                wqt = {}
                wkt = {}
                for hm in range(N_HB):
                    wqt[hm] = pool_wqk.tile([128, N_EB, 128], DT, tag="wqb",
                                            name=f"wqb{hm}")
                    wkt[hm] = pool_wqk.tile([128, N_EB, 128], DT, tag="wkb",
                                            name=f"wkb{hm}")

                # ------------- phase A2: v (resident, bf16) ----------------
                with (
                    tc.tile_pool(name="wv", bufs=1) as pool_wv,
                    tc.tile_pool(name="pv", bufs=4, space="PSUM") as psum_v,
                    tc.tile_pool(name="pd", bufs=1, space="PSUM") as psum_d,
                ):
                    # PE warmup: 3 fp32 matmuls (~1us each) on scratch. They
                    # depend only on the gpsimd memsets, so they run during
                    # the initial DMA and un-throttle the HAM clock gate.
                    dummy_ps = psum_d.tile([1, 512], F32, tag="dummy_ps",
                                           name="dummy_ps")
                    for i in range(3):
                        nc.tensor.matmul(dummy_ps[:], ones_f[:], dummy[:],
                                         start=True, stop=True)

                    wvh = [[pool_wv.tile([128, 512], DT, tag=f"wv{k}_{hc}",
                                         name=f"wv{k}_{hc}")
                            for hc in range(2)] for k in range(N_EB)]

                    def wv_dma(eng, k, hc):
                        eng.dma_start(
                            wvh[k][hc][:],
                            wvT_d[k * 128:(k + 1) * 128,
                                  hc * 512:(hc + 1) * 512])

                    # ---- sync ring: ramp share, then xf1a ----------------
                    wv_dma(nc.sync, 0, 0)
                    wv_dma(nc.sync, 0, 1)
                    nc.sync.dma_start(xf0[0][:], xT_d[0, :, 0:2, :])
                    wv_dma(nc.sync, 3, 0)
                    wv_dma(nc.sync, 3, 1)
                    nc.sync.dma_start(xf0[2][:], xT_d[0, :, 4:6, :])
                    wv_dma(nc.sync, 6, 0)
                    wv_dma(nc.sync, 6, 1)
                    nc.sync.dma_start(xf1[0][:], xT_d[1, :, 0:4, :])

                    # ---- scalar ring: ramp share, then xf1b --------------
                    wv_dma(nc.scalar, 1, 0)
                    wv_dma(nc.scalar, 1, 1)
                    nc.scalar.dma_start(xf0[1][:], xT_d[0, :, 2:4, :])
                    wv_dma(nc.scalar, 4, 0)
                    wv_dma(nc.scalar, 4, 1)
                    nc.scalar.dma_start(xf0[3][:], xT_d[0, :, 6:8, :])
                    wv_dma(nc.scalar, 7, 0)
                    wv_dma(nc.scalar, 7, 1)
                    nc.scalar.dma_start(xf1[1][:], xT_d[1, :, 4:8, :])

                    # ---- gpsimd ring: ramp share + W0/W1 + x chunks 2-3 --
                    wv_dma(nc.gpsimd, 2, 0)
                    wv_dma(nc.gpsimd, 2, 1)
                    wv_dma(nc.gpsimd, 5, 0)
                    wv_dma(nc.gpsimd, 5, 1)
                    nc.gpsimd.dma_start(wqt[0][:], wqT_d[0, :, :, :])
                    nc.gpsimd.dma_start(xf23[2][:], xT_d[2, :, :, :])
                    nc.gpsimd.dma_start(wkt[0][:], wkT_d[0, :, :, :])
                    nc.gpsimd.dma_start(xf23[3][:], xT_d[3, :, :, :])
                    nc.gpsimd.dma_start(wqt[1][:], wqT_d[1, :, :, :])
                    nc.gpsimd.dma_start(wkt[1][:], wkT_d[1, :, :, :])

                    with nc.named_scope("proj_v"):
                        for t8 in range(T // 256):
                            t4, half = t8 // 2, t8 % 2
                            if t8 == 0:
                                # ek-outer: 4 concurrent PSUM groups, so the
                                # DMA-paced ramp stalls in ~0.5MB steps
                                pvs = [psum_v.tile([128, 512], F32, tag="pv",
                                                   name=f"pv0_{g}")
                                       for g in range(4)]
                                for ek in range(N_EB):
                                    for ss in range(2):
                                        sl = slice(ss * 128, (ss + 1) * 128)
                                        for hc in range(2):
                                            nc.tensor.matmul(
                                                pvs[ss * 2 + hc][:],
                                                xf_slice(0, ek, sl),
                                                wvh[ek][hc][:],
                                                start=(ek == 0),
                                                stop=(ek == N_EB - 1))
                                for ss in range(2):
                                    for hc in range(2):
                                        dst = vt[ss][:, hc * 512:
                                                     (hc + 1) * 512]
                                        if hc == 0:
                                            nc.vector.tensor_copy(
                                                dst, pvs[ss * 2 + hc][:])
                                        else:
                                            nc.scalar.copy(
                                                dst, pvs[ss * 2 + hc][:])
                                continue
                            for ss in range(2):
                                j = t8 * 2 + ss
                                sl = slice(half * 256 + ss * 128,
                                           half * 256 + (ss + 1) * 128)
                                for hc in range(2):
                                    pv = psum_v.tile([128, 512], F32,
                                                     tag="pv",
                                                     name=f"pv_{t8}_{ss}_{hc}")
                                    for ek in range(N_EB):
                                        nc.tensor.matmul(
                                            pv[:], xf_slice(t4, ek, sl),
                                            wvh[ek][hc][:],
                                            start=(ek == 0),
                                            stop=(ek == N_EB - 1))
                                    dst = vt[j][:, hc * 512:(hc + 1) * 512]
                                    if hc == 0:
                                        nc.vector.tensor_copy(dst, pv[:])
                                    else:
                                        nc.scalar.copy(dst, pv[:])

                # ------------- phase A1: qT + kT (both resident) -----------
                with (
                    tc.tile_pool(name="kt", bufs=1) as pool_kt,
                    tc.tile_pool(name="qt", bufs=1) as pool_qt,
                ):
                    kt = [pool_kt.tile([128, T], BF, tag=f"kt{k}",
                                       name=f"kt{k}") for k in range(N_HB)]
                    qt = [pool_qt.tile([128, T], BF, tag=f"qt{k}",
                                       name=f"qt{k}") for k in range(N_HB)]

                    with tc.tile_pool(name="pa", bufs=4,
                                      space="PSUM") as psum_a:
                        with nc.named_scope("proj_qk"):
                            for hm in range(N_HB):
                                if hm + 2 < N_HB:
                                    # W stream rides the now-idle sync ring;
                                    # triggers gated by buffer rotation but
                                    # nothing critical queues behind them
                                    nc.sync.dma_start(wqt[hm + 2][:],
                                                      wqT_d[hm + 2, :, :, :])
                                    nc.sync.dma_start(wkt[hm + 2][:],
                                                      wkT_d[hm + 2, :, :, :])
                                for t4 in range(4):
                                    pq = psum_a.tile([128, 512], F32,
                                                     tag="pq",
                                                     name=f"pq_{hm}_{t4}")
                                    pk = psum_a.tile([128, 512], F32,
                                                     tag="pk",
                                                     name=f"pk_{hm}_{t4}")
                                    for ek in range(N_EB):
                                        nc.tensor.matmul(
                                            pq[:], wqt[hm][:, ek, :],
                                            xf_slice(t4, ek),
                                            start=(ek == 0),
                                            stop=(ek == N_EB - 1))
                                    for ek in range(N_EB):
                                        nc.tensor.matmul(
                                            pk[:], wkt[hm][:, ek, :],
                                            xf_slice(t4, ek),
                                            start=(ek == 0),
                                            stop=(ek == N_EB - 1))
                                    nc.scalar.copy(
                                        qt[hm][:, t4 * 512:(t4 + 1) * 512],
                                        pq[:])
                                    nc.vector.tensor_copy(
                                        kt[hm][:, t4 * 512:(t4 + 1) * 512],
                                        pk[:])

                    # ------------- phase B: causal attention ---------------
                    with (
                        tc.tile_pool(name="wt", bufs=4) as pool_wt,
                        tc.tile_pool(name="ob", bufs=6) as pool_ob,
                        tc.tile_pool(name="sm", bufs=4) as pool_sm,
                        tc.tile_pool(name="pb", bufs=1, space="PSUM") as psum_b,
                    ):
                        def scores(c, j):
                            n_j = 2 * c + 2
                            # last s-block: t-half 0 fully masked ->
                            # compute only the 128 t-half-1 columns
                            half = (j == n_j - 1)
                            off = 128 if half else 0
                            s_ps = psum_b.tile([128, TCB], F32,
                                               tag=f"S{j % 2}",
                                               name=f"S_{c}_{j}")
                            for hk in range(N_HB):
                                nc.tensor.matmul(
                                    s_ps[:, off:TCB],
                                    kt[hk][:, j * 128:(j + 1) * 128],
                                    qt[hk][:, c * TCB + off:(c + 1) * TCB],
                                    start=(hk == 0), stop=(hk == N_HB - 1))
                            wt = pool_wt.tile([128, TCB], BF, tag="wt",
                                              name=f"wt_{c}_{j}")
                            nc.scalar.activation(
                                wt[:, off:TCB], s_ps[:, off:TCB],
                                mybir.ActivationFunctionType.Exp,
                                scale=SCALE)
                            if j == 2 * c:
                                # diagonal block: t-half 0 triangular
                                nc.vector.tensor_mul(
                                    wt[:, 0:128], wt[:, 0:128], mask[:])
                            elif half:
                                # block j=2c+1: t-half 1 triangular
                                nc.vector.tensor_mul(
                                    wt[:, 128:TCB], wt[:, 128:TCB], mask[:])
                            return wt

                        def o_accum(c, j, wt, o_ps, rs_ps):
                            n_j = 2 * c + 2
                            first, last = (j == 0), (j == n_j - 1)
                            off = 128 if last else 0
                            nc.tensor.matmul(
                                rs_ps[0:1, off:TCB], ones_b[:],
                                wt[:, off:TCB],
                                start=first, stop=last,
                                skip_group_check=True)
                            for ts in range(2):
                                if ts == 0 and last:
                                    # fully masked: all-zero contribution
                                    continue
                                wslice = wt[:, ts * 128:(ts + 1) * 128]
                                last_ts = (j == n_j - 2) if ts == 0 else last
                                for hc in range(2):
                                    nc.tensor.matmul(
                                        o_ps[ts * 2 + hc][:], wslice,
                                        vt[j][:, hc * 512:(hc + 1) * 512],
                                        start=first, stop=last_ts)

                        def epilogue(c, o_ps, rs_ps):
                            rs_sb = pool_sm.tile([1, TCB], F32, tag="rs_sb",
                                                 name=f"rs_sb_{c}")
                            nc.vector.tensor_copy(rs_sb[:], rs_ps[:])
                            for ts in range(2):
                                # transpose [1,128]->[128,1] via matmul
                                rs_col = psum_b.tile([128, 1], F32,
                                                     tag="rs_col",
                                                     name=f"rs_col_{c}_{ts}")
                                nc.tensor.matmul(
                                    rs_col[:],
                                    rs_sb[0:1, ts * 128:(ts + 1) * 128],
                                    ones_f[0:1, 0:1],
                                    start=True, stop=True)
                                rec = pool_sm.tile([128, 1], F32, tag="rec",
                                                   name=f"rec_{c}_{ts}")
                                nc.vector.reciprocal(rec[:], rs_col[:])
                                for hc in range(2):
                                    ob = pool_ob.tile([128, 512], F32,
                                                      tag="ob",
                                                      name=f"ob_{c}_{ts}_{hc}")
                                    if hc == 0:
                                        nc.vector.tensor_scalar_mul(
                                            ob[:], o_ps[ts * 2 + hc][:],
                                            rec[:])
                                    else:
                                        nc.scalar.activation(
                                            ob[:], o_ps[ts * 2 + hc][:],
                                            mybir.ActivationFunctionType.Copy,
                                            scale=rec[:])
                                    out_ap = out_d[
                                        c * TCB + ts * 128:
                                        c * TCB + (ts + 1) * 128,
                                        hc * 512:(hc + 1) * 512]
                                    if c == N_TCB - 1:
                                        nc.sync.dma_start(out_ap, ob[:])
                                    else:
                                        nc.gpsimd.dma_start(out_ap, ob[:])

                        with nc.named_scope("attn"):
                            # software pipeline ACROSS chunks: scores of the
                            # next group issue before o_accum of the current
                            # one, so the exp chain and chunk epilogues hide
                            # under score matmuls
                            groups = [(c, j) for c in range(N_TCB)
                                      for j in range(2 * c + 2)]
                            chunk_ps = {}

                            def ensure_chunk(c):
                                if c not in chunk_ps:
                                    o_ps = [psum_b.tile([128, 512], F32,
                                                        tag=f"O{i}",
                                                        name=f"O_{c}_{i}")
                                            for i in range(4)]
                                    rs_ps = psum_b.tile([1, TCB], F32,
                                                        tag="rs",
                                                        name=f"rs_{c}")
                                    chunk_ps[c] = (o_ps, rs_ps)
                                return chunk_ps[c]

                            prev = None
                            for (c, j) in groups:
                                ensure_chunk(c)
                                wt_new = scores(c, j)
                                if prev is not None:
                                    pc, pj, pwt = prev
                                    po, prs = chunk_ps[pc]
                                    o_accum(pc, pj, pwt, po, prs)
                                    if pj == 2 * pc + 1:
                                        epilogue(pc, po, prs)
                                prev = (c, j, wt_new)
                            pc, pj, pwt = prev
                            po, prs = chunk_ps[pc]
                            o_accum(pc, pj, pwt, po, prs)
                            epilogue(pc, po, prs)

    nc.compile()
    return nc


_NC_CACHE = None


def _get_program():
    global _NC_CACHE
    if _NC_CACHE is None:
        _NC_CACHE = build_program()
    return _NC_CACHE


def make_in_maps(x, Wk, Wq, Wv):
    x = np.asarray(x, np.float32)
    xT = np.transpose(x, (0, 2, 1))                        # [B, E, T]
    # layout [t4][p][ek][512]: xT[e, t] with e = ek*128 + p
    xA = np.ascontiguousarray(
        xT.reshape(B, N_EB, 128, 4, 512).transpose(0, 3, 2, 1, 4))

    def prep_w(W):   # [H,E] -> W.T [E,H] -> [hm][p][ek][128]
        WT = np.asarray(W, np.float32).T
        return np.ascontiguousarray(
            WT.reshape(N_EB, 128, N_HB, 128).transpose(2, 1, 0, 3))

    WqT = prep_w(Wq)
    WkT = prep_w(Wk)
    WvT = np.ascontiguousarray(np.asarray(Wv, np.float32).T)  # [E, H]
    return [{"xA": xA[b], "WqT": WqT, "WkT": WkT, "WvT": WvT}
            for b in range(B)]


def kernel(x, Wk, Wq, Wv, _trace=False, _tmpdir=None):
    nc = _get_program()
    in_maps = make_in_maps(x, Wk, Wq, Wv)
    res = run_bass_kernel_spmd(nc, in_maps, list(range(N_CORES)),
                               trace=_trace, tmpdir=_tmpdir)
    out = np.stack([res.results[b]["out"] for b in range(B)])
    if _trace:
        kernel.last_result = res
    return out


# revision 11
# speedup vs baseline: 1.0161x; 1.0079x over previous
"""Single-head causal attention (B=8, T=2048, E=H=1024) on 8 TRN2 NeuronCores.

Strategy: data-parallel over batch (one batch element per core). Per core:
  warmup:   3 fp32 matmuls on a scratch tile keep the PE busy during the
            initial input DMA so the HAM clock-gate is already at 2.4 GHz
            when real work starts (cold PE runs at 1.2 GHz).
  phase A2: v = x@Wv.T [T,H] runs FIRST (x.T blocks stationary, Wv.T
            moving), resident in SBUF as bf16. x.T is read from the SAME
            resident tiles phase A1 uses — x is loaded from HBM exactly
            once (20MB total input DMA). The first leg runs ek-outer
            across 4 concurrent PSUM groups so the DMA-paced ramp makes
            sub-microsecond stalls instead of HAM-rethrottling gaps.
  phase A1: qT = (x@Wq.T).T and kT = (x@Wk.T).T  [H,T], contraction over
            E, hm outer / t4 inner (x.T resident, weights stream through
            2 rotating buffers). qT and kT are cast to bf16 and BOTH stay
            resident in SBUF — no DRAM spill.
  phase B:  causal flash attention over t-chunks of 256 in the S^T
            orientation: S^T[s,t] = sum_h kT[h,s]*qT[h,t] (bf16 operands,
            fp32 PSUM); softmax weights come out as wT[s_block, t] tiles
            feeding O[t,h] += wT.T @ v[s_block] directly. Row sums ride
            along as matmuls against a ones column. Diagonal masking uses
            one precomputed [128,128] bf16 triangle mask applied with a
            DVE multiply; the fully-masked t-half of the last s-block of
            each chunk is skipped (scores computed at N=128). The
            scores/accumulate software pipeline runs ACROSS chunk
            boundaries so the final diagonal exp chain and the epilogue
            overlap the next chunk's score matmuls.

DMA plan: a single HWDGE ring is trigger-rate-limited and any
semaphore-gated trigger blocks everything queued behind it, so the 20MB
load is split across all three rings (sync Q1, scalar-HWDGE Q10,
gpsimd-SWDGE Q0) in per-ring consumption order, first-leg tiles
interleaved across rings, W2..W7 gated-streamed on the idle sync ring
during A1.

Projections run in float32r (full-rate ~12-bit-significand fp32); the
attention part runs in bf16 (~4e-3 end-to-end vs the fp32 reference,
correctness gate is 2e-2).
"""

import numpy as np

import concourse.bacc as bacc
import concourse.mybir as mybir
import concourse.tile as tile
from concourse.bass_utils import run_bass_kernel_spmd

B, T, E, H = 8, 2048, 1024, 1024
N_CORES = 8
SCALE = float(E) ** -0.5

DT = mybir.dt.float32r
BF = mybir.dt.bfloat16
F32 = mybir.dt.float32

TCB = 256            # phase-B t-chunk width
N_TCB = T // TCB     # 8
N_EB = E // 128      # 8  e-blocks
N_HB = H // 128      # 8  h-blocks
N_SB = T // 128      # 16 s-blocks


def build_program():
    nc = bacc.Bacc("TRN2", target_bir_lowering=False, debug=False,
                   num_devices=N_CORES)

    # host-prepped layouts: every DMA reads long contiguous runs
    xT_d = nc.declare_dram_parameter("xA", [4, 128, N_EB, 512], DT,
                                     isOutput=False)   # [t4][p][ek][t]
    wqT_d = nc.declare_dram_parameter("WqT", [N_HB, 128, N_EB, 128], DT,
                                      isOutput=False)  # [hm][p][ek][h]
    wkT_d = nc.declare_dram_parameter("WkT", [N_HB, 128, N_EB, 128], DT,
                                      isOutput=False)
    wvT_d = nc.declare_dram_parameter("WvT", [E, H], DT, isOutput=False)
    out_d = nc.declare_dram_parameter("out", [T, H], BF, isOutput=True)

    with tile.TileContext(nc) as tc:
        with (
            tc.tile_pool(name="misc", bufs=1) as pool_misc,
            tc.tile_pool(name="v", bufs=1) as pool_v,
        ):
            vt = [pool_v.tile([128, H], BF, tag=f"v{j}", name=f"v{j}")
                  for j in range(N_SB)]

            ones_f = pool_misc.tile([128, 1], F32, tag="ones_f", name="ones_f")
            ones_b = pool_misc.tile([128, 1], BF, tag="ones_b", name="ones_b")
            mask = pool_misc.tile([128, 128], BF, tag="mask", name="mask")
            dummy = pool_misc.tile([128, 512], F32, tag="dummy", name="dummy")
            nc.gpsimd.memset(ones_f[:], 1.0)
            nc.gpsimd.memset(dummy[:], 0.0)
            nc.gpsimd.memset(mask[:], 1.0)
            nc.vector.tensor_copy(ones_b[:], ones_f[:])
            # triangle mask: keep s <= t within a 128x128 block
            nc.gpsimd.affine_select(
                out=mask[:], in_=mask[:],
                compare_op=mybir.AluOpType.is_ge,
                fill=0.0, base=0, channel_multiplier=-1,
                pattern=[[1, 128]])

            with (
                tc.tile_pool(name="xf", bufs=1) as pool_xf,
                tc.tile_pool(name="wqk", bufs=2) as pool_wqk,
            ):
                # x chunks 0 and 1 are split so the ramp waits on small
                # pieces; chunks 2-3 land long before use
                xf0 = [pool_xf.tile([128, 2, 512], DT, tag=f"xf0_{i}",
                                    name=f"xf0_{i}") for i in range(4)]
                xf1 = [pool_xf.tile([128, 4, 512], DT, tag=f"xf1_{i}",
                                    name=f"xf1_{i}") for i in range(2)]
                xf23 = {t4: pool_xf.tile([128, N_EB, 512], DT, tag=f"xf{t4}",
                                         name=f"xf{t4}") for t4 in (2, 3)}

                def xf_slice(t4, ek, sl=slice(0, 512)):
                    if t4 == 0:
                        return xf0[ek // 2][:, ek % 2, sl]
                    if t4 == 1:
                        return xf1[ek // 4][:, ek % 4, sl]
                    return xf23[t4][:, ek, sl]

                wqt = {}
                wkt = {}
                for hm in range(N_HB):
                    wqt[hm] = pool_wqk.tile([128, N_EB, 128], DT, tag="wqb",
                                            name=f"wqb{hm}")
                    wkt[hm] = pool_wqk.tile([128, N_EB, 128], DT, tag="wkb",
                                            name=f"wkb{hm}")

                # ------------- phase A2: v (resident, bf16) ----------------
                with (
                    tc.tile_pool(name="wv", bufs=1) as pool_wv,
                    tc.tile_pool(name="pv", bufs=4, space="PSUM") as psum_v,
                    tc.tile_pool(name="pd", bufs=1, space="PSUM") as psum_d,
                ):
                    # PE warmup: 3 fp32 matmuls (~1us each) on scratch. They
                    # depend only on the gpsimd memsets, so they run during
                    # the initial DMA and un-throttle the HAM clock gate.
                    dummy_ps = psum_d.tile([1, 512], F32, tag="dummy_ps",
                                           name="dummy_ps")
                    for i in range(3):
                        nc.tensor.matmul(dummy_ps[:], ones_f[:], dummy[:],
                                         start=True, stop=True)

                    # Wv as 4x 1MB tiles (ek-pairs): each HWDGE ring only
                    # supports ~4 outstanding DMAs with completion-gated
                    # trigger reuse, so transfers must be big to keep the
                    # SDMA engines fed
                    wv_q = [pool_wv.tile([128, 2, 1024], DT, tag=f"wvq{i}",
                                         name=f"wvq{i}") for i in range(4)]

                    def wvh(k, hc):
                        return wv_q[k // 2][:, k % 2,
                                            hc * 512:(hc + 1) * 512]

                    wv_src = wvT_d.rearrange("(k p) h -> p k h", p=128)

                    # ---- sync ring ---------------------------------------
                    nc.sync.dma_start(wv_q[0][:], wv_src[:, 0:2, :])
                    nc.sync.dma_start(xf0[1][:], xT_d[0, :, 2:4, :])
                    nc.sync.dma_start(wv_q[2][:], wv_src[:, 4:6, :])
                    nc.sync.dma_start(xf0[3][:], xT_d[0, :, 6:8, :])
                    nc.sync.dma_start(xf1[0][:], xT_d[1, :, 0:4, :])
                    nc.sync.dma_start(xf1[1][:], xT_d[1, :, 4:8, :])

                    # ---- scalar ring -------------------------------------
                    nc.scalar.dma_start(xf0[0][:], xT_d[0, :, 0:2, :])
                    nc.scalar.dma_start(wv_q[1][:], wv_src[:, 2:4, :])
                    nc.scalar.dma_start(xf0[2][:], xT_d[0, :, 4:6, :])
                    nc.scalar.dma_start(wv_q[3][:], wv_src[:, 6:8, :])
                    nc.scalar.dma_start(xf23[2][:], xT_d[2, :, :, :])

                    # ---- gpsimd ring: W0/W1 + x chunk 3 ------------------
                    nc.gpsimd.dma_start(wqt[0][:], wqT_d[0, :, :, :])
                    nc.gpsimd.dma_start(wkt[0][:], wkT_d[0, :, :, :])
                    nc.gpsimd.dma_start(xf23[3][:], xT_d[3, :, :, :])
                    nc.gpsimd.dma_start(wqt[1][:], wqT_d[1, :, :, :])
                    nc.gpsimd.dma_start(wkt[1][:], wkT_d[1, :, :, :])

                    with nc.named_scope("proj_v"):
                        for t8 in range(T // 256):
                            t4, half = t8 // 2, t8 % 2
                            if t8 == 0:
                                # ek-outer: 4 concurrent PSUM groups, so the
                                # DMA-paced ramp stalls in ~0.5MB steps
                                pvs = [psum_v.tile([128, 512], F32, tag="pv",
                                                   name=f"pv0_{g}")
                                       for g in range(4)]
                                for ek in range(N_EB):
                                    for ss in range(2):
                                        sl = slice(ss * 128, (ss + 1) * 128)
                                        for hc in range(2):
                                            nc.tensor.matmul(
                                                pvs[ss * 2 + hc][:],
                                                xf_slice(0, ek, sl),
                                                wvh(ek, hc),
                                                start=(ek == 0),
                                                stop=(ek == N_EB - 1))
                                for ss in range(2):
                                    for hc in range(2):
                                        dst = vt[ss][:, hc * 512:
                                                     (hc + 1) * 512]
                                        if hc == 0:
                                            nc.vector.tensor_copy(
                                                dst, pvs[ss * 2 + hc][:])
                                        else:
                                            nc.scalar.copy(
                                                dst, pvs[ss * 2 + hc][:])
                                continue
                            for ss in range(2):
                                j = t8 * 2 + ss
                                sl = slice(half * 256 + ss * 128,
                                           half * 256 + (ss + 1) * 128)
                                for hc in range(2):
                                    pv = psum_v.tile([128, 512], F32,
                                                     tag="pv",
                                                     name=f"pv_{t8}_{ss}_{hc}")
                                    for ek in range(N_EB):
                                        nc.tensor.matmul(
                                            pv[:], xf_slice(t4, ek, sl),
                                            wvh(ek, hc),
                                            start=(ek == 0),
                                            stop=(ek == N_EB - 1))
                                    dst = vt[j][:, hc * 512:(hc + 1) * 512]
                                    if hc == 0:
                                        nc.vector.tensor_copy(dst, pv[:])
                                    else:
                                        nc.scalar.copy(dst, pv[:])

                # ------------- phase A1: qT + kT (both resident) -----------
                with (
                    tc.tile_pool(name="kt", bufs=1) as pool_kt,
                    tc.tile_pool(name="qt", bufs=1) as pool_qt,
                ):
                    kt = [pool_kt.tile([128, T], BF, tag=f"kt{k}",
                                       name=f"kt{k}") for k in range(N_HB)]
                    qt = [pool_qt.tile([128, T], BF, tag=f"qt{k}",
                                       name=f"qt{k}") for k in range(N_HB)]

                    with tc.tile_pool(name="pa", bufs=4,
                                      space="PSUM") as psum_a:
                        with nc.named_scope("proj_qk"):
                            for hm in range(N_HB):
                                if hm + 2 < N_HB:
                                    # W stream rides the now-idle sync ring;
                                    # triggers gated by buffer rotation but
                                    # nothing critical queues behind them
                                    nc.sync.dma_start(wqt[hm + 2][:],
                                                      wqT_d[hm + 2, :, :, :])
                                    nc.sync.dma_start(wkt[hm + 2][:],
                                                      wkT_d[hm + 2, :, :, :])
                                for t4 in range(4):
                                    pq = psum_a.tile([128, 512], F32,
                                                     tag="pq",
                                                     name=f"pq_{hm}_{t4}")
                                    pk = psum_a.tile([128, 512], F32,
                                                     tag="pk",
                                                     name=f"pk_{hm}_{t4}")
                                    for ek in range(N_EB):
                                        nc.tensor.matmul(
                                            pq[:], wqt[hm][:, ek, :],
                                            xf_slice(t4, ek),
                                            start=(ek == 0),
                                            stop=(ek == N_EB - 1))
                                    for ek in range(N_EB):
                                        nc.tensor.matmul(
                                            pk[:], wkt[hm][:, ek, :],
                                            xf_slice(t4, ek),
                                            start=(ek == 0),
                                            stop=(ek == N_EB - 1))
                                    nc.scalar.copy(
                                        qt[hm][:, t4 * 512:(t4 + 1) * 512],
                                        pq[:])
                                    nc.vector.tensor_copy(
                                        kt[hm][:, t4 * 512:(t4 + 1) * 512],
                                        pk[:])

                    # ------------- phase B: causal attention ---------------
                    with (
                        tc.tile_pool(name="wt", bufs=4) as pool_wt,
                        tc.tile_pool(name="ob", bufs=6) as pool_ob,
                        tc.tile_pool(name="sm", bufs=4) as pool_sm,
                        tc.tile_pool(name="pb", bufs=1, space="PSUM") as psum_b,
                    ):
                        def scores(c, j):
                            n_j = 2 * c + 2
                            # last s-block: t-half 0 fully masked ->
                            # compute only the 128 t-half-1 columns
                            half = (j == n_j - 1)
                            off = 128 if half else 0
                            s_ps = psum_b.tile([128, TCB], F32,
                                               tag=f"S{j % 2}",
                                               name=f"S_{c}_{j}")
                            for hk in range(N_HB):
                                nc.tensor.matmul(
                                    s_ps[:, off:TCB],
                                    kt[hk][:, j * 128:(j + 1) * 128],
                                    qt[hk][:, c * TCB + off:(c + 1) * TCB],
                                    start=(hk == 0), stop=(hk == N_HB - 1))
                            wt = pool_wt.tile([128, TCB], BF, tag="wt",
                                              name=f"wt_{c}_{j}")
                            nc.scalar.activation(
                                wt[:, off:TCB], s_ps[:, off:TCB],
                                mybir.ActivationFunctionType.Exp,
                                scale=SCALE)
                            if j == 2 * c:
                                # diagonal block: t-half 0 triangular
                                nc.vector.tensor_mul(
                                    wt[:, 0:128], wt[:, 0:128], mask[:])
                            elif half:
                                # block j=2c+1: t-half 1 triangular
                                nc.vector.tensor_mul(
                                    wt[:, 128:TCB], wt[:, 128:TCB], mask[:])
                            return wt

                        def o_accum(c, j, wt, o_ps, rs_ps):
                            n_j = 2 * c + 2
                            first, last = (j == 0), (j == n_j - 1)
                            off = 128 if last else 0
                            nc.tensor.matmul(
                                rs_ps[0:1, off:TCB], ones_b[:],
                                wt[:, off:TCB],
                                start=first, stop=last,
                                skip_group_check=True)
                            for ts in range(2):
                                if ts == 0 and last:
                                    # fully masked: all-zero contribution
                                    continue
                                wslice = wt[:, ts * 128:(ts + 1) * 128]
                                last_ts = (j == n_j - 2) if ts == 0 else last
                                for hc in range(2):
                                    nc.tensor.matmul(
                                        o_ps[ts * 2 + hc][:], wslice,
                                        vt[j][:, hc * 512:(hc + 1) * 512],
                                        start=first, stop=last_ts)

                        def epilogue(c, o_ps, rs_ps):
                            rs_sb = pool_sm.tile([1, TCB], F32, tag="rs_sb",
                                                 name=f"rs_sb_{c}")
                            nc.vector.tensor_copy(rs_sb[:], rs_ps[:])
                            # both transposes first (PE), then both recips
                            # (DVE), then the 4 normalizes on alternating
                            # engines -- shortest serial chain
                            rs_col = psum_b.tile([128, 2], F32, tag="rs_col",
                                                 name=f"rs_col_{c}")
                            for ts in range(2):
                                # transpose [1,128]->[128,1] via matmul;
                                # second write must not clear the first
                                # column, so accumulate-mode with the
                                # has_written bits doing the overwrite
                                nc.tensor.matmul(
                                    rs_col[:, ts:ts + 1],
                                    rs_sb[0:1, ts * 128:(ts + 1) * 128],
                                    ones_f[0:1, 0:1],
                                    start=(ts == 0), stop=(ts == 1),
                                    skip_group_check=True)
                            rec = pool_sm.tile([128, 2], F32, tag="rec",
                                               name=f"rec_{c}")
                            nc.vector.reciprocal(rec[:], rs_col[:])
                            for ts in range(2):
                                for hc in range(2):
                                    ob = pool_ob.tile([128, 512], BF,
                                                      tag="ob",
                                                      name=f"ob_{c}_{ts}_{hc}")
                                    if hc == 0:
                                        nc.vector.tensor_scalar_mul(
                                            ob[:], o_ps[ts * 2 + hc][:],
                                            rec[:, ts:ts + 1])
                                    else:
                                        nc.scalar.activation(
                                            ob[:], o_ps[ts * 2 + hc][:],
                                            mybir.ActivationFunctionType.Copy,
                                            scale=rec[:, ts:ts + 1])
                                    out_ap = out_d[
                                        c * TCB + ts * 128:
                                        c * TCB + (ts + 1) * 128,
                                        hc * 512:(hc + 1) * 512]
                                    if c == N_TCB - 1:
                                        nc.sync.dma_start(out_ap, ob[:])
                                    else:
                                        nc.gpsimd.dma_start(out_ap, ob[:])

                        with nc.named_scope("attn"):
                            # software pipeline ACROSS chunks: scores of the
                            # next group issue before o_accum of the current
                            # one, so the exp chain and chunk epilogues hide
                            # under score matmuls
                            groups = [(c, j) for c in range(N_TCB)
                                      for j in range(2 * c + 2)]
                            chunk_ps = {}

                            def ensure_chunk(c):
                                if c not in chunk_ps:
                                    o_ps = [psum_b.tile([128, 512], F32,
                                                        tag=f"O{i}",
                                                        name=f"O_{c}_{i}")
                                            for i in range(4)]
                                    rs_ps = psum_b.tile([1, TCB], F32,
                                                        tag="rs",
                                                        name=f"rs_{c}")
                                    chunk_ps[c] = (o_ps, rs_ps)
                                return chunk_ps[c]

                            prev = None
                            for (c, j) in groups:
                                ensure_chunk(c)
                                wt_new = scores(c, j)
                                if prev is not None:
                                    pc, pj, pwt = prev
                                    po, prs = chunk_ps[pc]
                                    o_accum(pc, pj, pwt, po, prs)
                                    if pj == 2 * pc + 1:
                                        epilogue(pc, po, prs)
                                prev = (c, j, wt_new)
                            pc, pj, pwt = prev
                            po, prs = chunk_ps[pc]
                            o_accum(pc, pj, pwt, po, prs)
                            epilogue(pc, po, prs)

    nc.compile()
    return nc


_NC_CACHE = None


def _get_program():
    global _NC_CACHE
    if _NC_CACHE is None:
        _NC_CACHE = build_program()
    return _NC_CACHE


def make_in_maps(x, Wk, Wq, Wv):
    x = np.asarray(x, np.float32)
    xT = np.transpose(x, (0, 2, 1))                        # [B, E, T]
    # layout [t4][p][ek][512]: xT[e, t] with e = ek*128 + p
    xA = np.ascontiguousarray(
        xT.reshape(B, N_EB, 128, 4, 512).transpose(0, 3, 2, 1, 4))

    def prep_w(W):   # [H,E] -> W.T [E,H] -> [hm][p][ek][128]
        WT = np.asarray(W, np.float32).T
        return np.ascontiguousarray(
            WT.reshape(N_EB, 128, N_HB, 128).transpose(2, 1, 0, 3))

    WqT = prep_w(Wq)
    WkT = prep_w(Wk)
    WvT = np.ascontiguousarray(np.asarray(Wv, np.float32).T)  # [E, H]
    return [{"xA": xA[b], "WqT": WqT, "WkT": WkT, "WvT": WvT}
            for b in range(B)]


def kernel(x, Wk, Wq, Wv, _trace=False, _tmpdir=None):
    nc = _get_program()
    in_maps = make_in_maps(x, Wk, Wq, Wv)
    res = run_bass_kernel_spmd(nc, in_maps, list(range(N_CORES)),
                               trace=_trace, tmpdir=_tmpdir)
    out = np.stack([np.asarray(res.results[b]["out"]) for b in range(B)])
    out = out.astype(np.float32)
    if _trace:
        kernel.last_result = res
    return out


# revision 15
# speedup vs baseline: 1.1643x; 1.1459x over previous
"""Single-head causal attention (B=8, T=2048, E=H=1024) on 8 TRN2 NeuronCores.

Strategy: data-parallel over batch (one batch element per core). The whole
kernel runs in bf16 operands with fp32 PSUM accumulation (measured ~8e-3
max-rel vs the fp32 reference; the correctness gate is 2e-2). bf16 halves
every DMA and SBUF footprint: total input traffic is 10MB/core.

Per core:
  warmup:   3 fp32 matmuls on a scratch tile keep the PE busy during the
            initial input DMA so the HAM clock-gate is already at 2.4 GHz
            when real work starts (cold PE runs at 1.2 GHz).
  phase A2: v = x@Wv.T [T,H] runs FIRST (x.T blocks stationary, Wv.T
            moving), resident in SBUF. Its ramp needs only 3MB (Wv 2MB +
            first x chunk 1MB); the first leg runs ek-outer across 4
            concurrent PSUM groups so the DMA-paced ramp makes small
            stalls instead of HAM-rethrottling gaps. All remaining input
            prefetches behind the ramp on the three DMA rings.
  phase A1: qT = (x@Wq.T).T and kT = (x@Wk.T).T  [H,T], contraction over
            E, hm outer / t4 inner (x.T resident, weights stream through
            2 rotating buffers). qT and kT both stay resident in SBUF.
  phase B:  causal flash attention over t-chunks of 256 in the S^T
            orientation: S^T[s,t] = sum_h kT[h,s]*qT[h,t], softmax weights
            come out as wT[s_block, t] tiles feeding O[t,h] += wT.T @
            v[s_block] directly. Row sums ride along as matmuls against a
            ones column, issued AFTER the O matmuls so the weight-load
            pipeline between score- and O-groups stays full. Diagonal
            masking uses one precomputed [128,128] bf16 triangle mask
            applied with a DVE multiply; the fully-masked t-half of the
            last s-block of each chunk is skipped (scores at N=128). The
            scores/accumulate software pipeline runs ACROSS chunk
            boundaries so the final diagonal exp chain and the epilogue
            overlap the next chunk's score matmuls. The row-sum transpose
            matmuls use float32r operands (single-pass on the PE; true
            fp32 lowers to a LOW/HIGH pair, 4x the cost).

DMA plan: within a ring DMAs complete in FIFO order; rings round-robin
for SDMA engine bandwidth, and each HWDGE ring allows ~4 outstanding
DMAs with completion-gated trigger reuse. So: big transfers, per-ring
consumption order, the 3MB ramp spread across all three rings, and
everything else enqueued behind it.
"""

import numpy as np
import ml_dtypes

import concourse.bacc as bacc
import concourse.mybir as mybir
import concourse.tile as tile
from concourse.bass_utils import run_bass_kernel_spmd

B, T, E, H = 8, 2048, 1024, 1024
N_CORES = 8
SCALE = float(E) ** -0.5

DT = mybir.dt.float32r
BF = mybir.dt.bfloat16
F32 = mybir.dt.float32

TCB = 256            # phase-B t-chunk width
N_TCB = T // TCB     # 8
N_EB = E // 128      # 8  e-blocks
N_HB = H // 128      # 8  h-blocks
N_SB = T // 128      # 16 s-blocks


def build_program():
    nc = bacc.Bacc("TRN2", target_bir_lowering=False, debug=False,
                   num_devices=N_CORES)

    # host-prepped layouts (all bf16): every DMA reads contiguous runs
    xT_d = nc.declare_dram_parameter("xA", [4, 128, N_EB, 512], BF,
                                     isOutput=False)   # [t4][p][ek][t]
    wqT_d = nc.declare_dram_parameter("WqT", [N_HB, 128, N_EB, 128], BF,
                                      isOutput=False)  # [hm][p][ek][h]
    wkT_d = nc.declare_dram_parameter("WkT", [N_HB, 128, N_EB, 128], BF,
                                      isOutput=False)
    wvT_d = nc.declare_dram_parameter("WvT", [E, H], BF, isOutput=False)
    out_d = nc.declare_dram_parameter("out", [T, H], BF, isOutput=True)

    with tile.TileContext(nc) as tc:
        with (
            tc.tile_pool(name="misc", bufs=1) as pool_misc,
            tc.tile_pool(name="v", bufs=1) as pool_v,
        ):
            vt = [pool_v.tile([128, H], BF, tag=f"v{j}", name=f"v{j}")
                  for j in range(N_SB)]

            ones_f = pool_misc.tile([128, 1], F32, tag="ones_f", name="ones_f")
            ones_b = pool_misc.tile([128, 1], BF, tag="ones_b", name="ones_b")
            mask = pool_misc.tile([128, 128], BF, tag="mask", name="mask")
            dummy = pool_misc.tile([128, 512], F32, tag="dummy", name="dummy")
            nc.gpsimd.memset(ones_f[:], 1.0)
            nc.vector.memset(dummy[:], 0.0)
            nc.gpsimd.memset(mask[:], 1.0)
            nc.vector.tensor_copy(ones_b[:], ones_f[:])
            # triangle mask: keep s <= t within a 128x128 block
            nc.gpsimd.affine_select(
                out=mask[:], in_=mask[:],
                compare_op=mybir.AluOpType.is_ge,
                fill=0.0, base=0, channel_multiplier=-1,
                pattern=[[1, 128]])

            with (
                tc.tile_pool(name="xf", bufs=1) as pool_xf,
                tc.tile_pool(name="wqk", bufs=2) as pool_wqk,
            ):
                # x chunk 0 split in half for a fast ramp
                xf0 = [pool_xf.tile([128, 4, 512], BF, tag=f"xf0_{i}",
                                    name=f"xf0_{i}") for i in range(2)]
                xf123 = {t4: pool_xf.tile([128, N_EB, 512], BF,
                                          tag=f"xf{t4}", name=f"xf{t4}")
                         for t4 in (1, 2, 3)}

                def xf_slice(t4, ek, sl=slice(0, 512)):
                    if t4 == 0:
                        return xf0[ek // 4][:, ek % 4, sl]
                    return xf123[t4][:, ek, sl]

                wqt = {}
                wkt = {}
                for hm in range(N_HB):
                    wqt[hm] = pool_wqk.tile([128, N_EB, 128], BF, tag="wqb",
                                            name=f"wqb{hm}")
                    wkt[hm] = pool_wqk.tile([128, N_EB, 128], BF, tag="wkb",
                                            name=f"wkb{hm}")

                # ------------- phase A2: v (resident) ----------------------
                with (
                    tc.tile_pool(name="wv", bufs=1) as pool_wv,
                    tc.tile_pool(name="pv", bufs=4, space="PSUM") as psum_v,
                    tc.tile_pool(name="pd", bufs=1, space="PSUM") as psum_d,
                ):
                    # PE warmup on scratch (no input dependency)
                    dummy_ps = psum_d.tile([1, 512], F32, tag="dummy_ps",
                                           name="dummy_ps")
                    for i in range(3):
                        nc.tensor.matmul(dummy_ps[:], ones_f[:], dummy[:],
                                         start=True, stop=True)

                    wv_q = [pool_wv.tile([128, 2, 1024], BF, tag=f"wvq{i}",
                                         name=f"wvq{i}") for i in range(4)]

                    def wvh(k, hc):
                        return wv_q[k // 2][:, k % 2,
                                            hc * 512:(hc + 1) * 512]

                    wv_src = wvT_d.rearrange("(k p) h -> p k h", p=128)

                    # ramp (3MB) spread across the three rings in
                    # consumption order; everything else queues behind
                    # ---- sync ring ---------------------------------------
                    nc.sync.dma_start(wv_q[0][:], wv_src[:, 0:2, :])
                    nc.sync.dma_start(xf0[1][:], xT_d[0, :, 4:8, :])
                    nc.sync.dma_start(wv_q[2][:], wv_src[:, 4:6, :])
                    # ---- scalar ring -------------------------------------
                    nc.scalar.dma_start(xf0[0][:], xT_d[0, :, 0:4, :])
                    nc.scalar.dma_start(wv_q[1][:], wv_src[:, 2:4, :])
                    nc.scalar.dma_start(wv_q[3][:], wv_src[:, 6:8, :])
                    # ---- gpsimd ring: the whole A1 prefetch --------------
                    nc.gpsimd.dma_start(xf123[1][:], xT_d[1, :, :, :])
                    nc.gpsimd.dma_start(wqt[0][:], wqT_d[0, :, :, :])
                    nc.gpsimd.dma_start(wkt[0][:], wkT_d[0, :, :, :])
                    nc.gpsimd.dma_start(xf123[2][:], xT_d[2, :, :, :])
                    nc.gpsimd.dma_start(xf123[3][:], xT_d[3, :, :, :])
                    nc.gpsimd.dma_start(wqt[1][:], wqT_d[1, :, :, :])
                    nc.gpsimd.dma_start(wkt[1][:], wkT_d[1, :, :, :])

                    with nc.named_scope("proj_v"):
                        for t8 in range(T // 256):
                            t4, half = t8 // 2, t8 % 2
                            if t8 == 0:
                                # ek-outer: 4 concurrent PSUM groups, so the
                                # DMA-paced ramp stalls in small steps
                                pvs = [psum_v.tile([128, 512], F32, tag="pv",
                                                   name=f"pv0_{g}")
                                       for g in range(4)]
                                for ek in range(N_EB):
                                    for ss in range(2):
                                        sl = slice(ss * 128, (ss + 1) * 128)
                                        for hc in range(2):
                                            nc.tensor.matmul(
                                                pvs[ss * 2 + hc][:],
                                                xf_slice(0, ek, sl),
                                                wvh(ek, hc),
                                                start=(ek == 0),
                                                stop=(ek == N_EB - 1))
                                for ss in range(2):
                                    for hc in range(2):
                                        dst = vt[ss][:, hc * 512:
                                                     (hc + 1) * 512]
                                        if hc == 0:
                                            nc.vector.tensor_copy(
                                                dst, pvs[ss * 2 + hc][:])
                                        else:
                                            nc.scalar.copy(
                                                dst, pvs[ss * 2 + hc][:])
                                continue
                            for ss in range(2):
                                j = t8 * 2 + ss
                                sl = slice(half * 256 + ss * 128,
                                           half * 256 + (ss + 1) * 128)
                                for hc in range(2):
                                    pv = psum_v.tile([128, 512], F32,
                                                     tag="pv",
                                                     name=f"pv_{t8}_{ss}_{hc}")
                                    for ek in range(N_EB):
                                        nc.tensor.matmul(
                                            pv[:], xf_slice(t4, ek, sl),
                                            wvh(ek, hc),
                                            start=(ek == 0),
                                            stop=(ek == N_EB - 1))
                                    dst = vt[j][:, hc * 512:(hc + 1) * 512]
                                    if hc == 0:
                                        nc.vector.tensor_copy(dst, pv[:])
                                    else:
                                        nc.scalar.copy(dst, pv[:])

                # ------------- phase A1: qT + kT (both resident) -----------
                with (
                    tc.tile_pool(name="kt", bufs=1) as pool_kt,
                    tc.tile_pool(name="qt", bufs=1) as pool_qt,
                ):
                    kt = [pool_kt.tile([128, T], BF, tag=f"kt{k}",
                                       name=f"kt{k}") for k in range(N_HB)]
                    qt = [pool_qt.tile([128, T], BF, tag=f"qt{k}",
                                       name=f"qt{k}") for k in range(N_HB)]

                    with tc.tile_pool(name="pa", bufs=4,
                                      space="PSUM") as psum_a:
                        with nc.named_scope("proj_qk"):
                            for hm in range(N_HB):
                                if hm + 2 < N_HB:
                                    # W stream rides the now-idle sync ring
                                    nc.sync.dma_start(wqt[hm + 2][:],
                                                      wqT_d[hm + 2, :, :, :])
                                    nc.sync.dma_start(wkt[hm + 2][:],
                                                      wkT_d[hm + 2, :, :, :])
                                for t4 in range(4):
                                    pq = psum_a.tile([128, 512], F32,
                                                     tag="pq",
                                                     name=f"pq_{hm}_{t4}")
                                    pk = psum_a.tile([128, 512], F32,
                                                     tag="pk",
                                                     name=f"pk_{hm}_{t4}")
                                    for ek in range(N_EB):
                                        nc.tensor.matmul(
                                            pq[:], wqt[hm][:, ek, :],
                                            xf_slice(t4, ek),
                                            start=(ek == 0),
                                            stop=(ek == N_EB - 1))
                                    for ek in range(N_EB):
                                        nc.tensor.matmul(
                                            pk[:], wkt[hm][:, ek, :],
                                            xf_slice(t4, ek),
                                            start=(ek == 0),
                                            stop=(ek == N_EB - 1))
                                    nc.scalar.copy(
                                        qt[hm][:, t4 * 512:(t4 + 1) * 512],
                                        pq[:])
                                    nc.vector.tensor_copy(
                                        kt[hm][:, t4 * 512:(t4 + 1) * 512],
                                        pk[:])

                    # ------------- phase B: causal attention ---------------
                    with (
                        tc.tile_pool(name="wt", bufs=4) as pool_wt,
                        tc.tile_pool(name="ob", bufs=6) as pool_ob,
                        tc.tile_pool(name="sm", bufs=4) as pool_sm,
                        tc.tile_pool(name="pb", bufs=1, space="PSUM") as psum_b,
                    ):
                        def scores(c, j):
                            n_j = 2 * c + 2
                            # last s-block: t-half 0 fully masked ->
                            # compute only the 128 t-half-1 columns
                            half = (j == n_j - 1)
                            off = 128 if half else 0
                            s_ps = psum_b.tile([128, TCB], F32,
                                               tag=f"S{j % 2}",
                                               name=f"S_{c}_{j}")
                            for hk in range(N_HB):
                                nc.tensor.matmul(
                                    s_ps[:, off:TCB],
                                    kt[hk][:, j * 128:(j + 1) * 128],
                                    qt[hk][:, c * TCB + off:(c + 1) * TCB],
                                    start=(hk == 0), stop=(hk == N_HB - 1))
                            wt = pool_wt.tile([128, TCB], BF, tag="wt",
                                              name=f"wt_{c}_{j}")
                            nc.scalar.activation(
                                wt[:, off:TCB], s_ps[:, off:TCB],
                                mybir.ActivationFunctionType.Exp,
                                scale=SCALE)
                            if j == 2 * c:
                                # diagonal block: t-half 0 triangular
                                nc.vector.tensor_mul(
                                    wt[:, 0:128], wt[:, 0:128], mask[:])
                            elif half:
                                # block j=2c+1: t-half 1 triangular
                                nc.vector.tensor_mul(
                                    wt[:, 128:TCB], wt[:, 128:TCB], mask[:])
                            return wt

                        def o_accum(c, j, wt, o_ps, rs_ps):
                            n_j = 2 * c + 2
                            first, last = (j == 0), (j == n_j - 1)
                            for ts in range(2):
                                if ts == 0 and last:
                                    # fully masked: all-zero contribution
                                    continue
                                wslice = wt[:, ts * 128:(ts + 1) * 128]
                                last_ts = (j == n_j - 2) if ts == 0 else last
                                for hc in range(2):
                                    nc.tensor.matmul(
                                        o_ps[ts * 2 + hc][:], wslice,
                                        vt[j][:, hc * 512:(hc + 1) * 512],
                                        start=first, stop=last_ts)
                                # row-sum directly in [t,1] orientation:
                                # stationary wt-slice (just loaded for the O
                                # matmuls), moving ones column. No transpose
                                # needed anywhere.
                                nc.tensor.matmul(
                                    rs_ps[ts][:], wslice, ones_b[:],
                                    start=first, stop=last_ts)

                        def epilogue(c, o_ps, rs_ps):
                            rec = pool_sm.tile([128, 2], F32, tag="rec",
                                               name=f"rec_{c}")
                            nc.vector.reciprocal(rec[:, 0:1], rs_ps[0][:])
                            nc.vector.reciprocal(rec[:, 1:2], rs_ps[1][:])
                            for ts in range(2):
                                for hc in range(2):
                                    ob = pool_ob.tile([128, 512], BF,
                                                      tag="ob",
                                                      name=f"ob_{c}_{ts}_{hc}")
                                    if hc == 0:
                                        nc.vector.tensor_scalar_mul(
                                            ob[:], o_ps[ts * 2 + hc][:],
                                            rec[:, ts:ts + 1])
                                    else:
                                        nc.scalar.activation(
                                            ob[:], o_ps[ts * 2 + hc][:],
                                            mybir.ActivationFunctionType.Copy,
                                            scale=rec[:, ts:ts + 1])
                                    out_ap = out_d[
                                        c * TCB + ts * 128:
                                        c * TCB + (ts + 1) * 128,
                                        hc * 512:(hc + 1) * 512]
                                    if c == N_TCB - 1:
                                        nc.sync.dma_start(out_ap, ob[:])
                                    else:
                                        nc.gpsimd.dma_start(out_ap, ob[:])

                        with nc.named_scope("attn"):
                            # software pipeline ACROSS chunks: scores of the
                            # next group issue before o_accum of the current
                            # one, so the exp chain and chunk epilogues hide
                            # under score matmuls
                            groups = [(c, j) for c in range(N_TCB)
                                      for j in range(2 * c + 2)]
                            chunk_ps = {}

                            def ensure_chunk(c):
                                if c not in chunk_ps:
                                    o_ps = [psum_b.tile([128, 512], F32,
                                                        tag=f"O{i}",
                                                        name=f"O_{c}_{i}")
                                            for i in range(4)]
                                    rs_ps = [psum_b.tile([128, 1], F32,
                                                         tag=f"rsT{t}",
                                                         name=f"rs_{c}_{t}")
                                             for t in range(2)]
                                    chunk_ps[c] = (o_ps, rs_ps)
                                return chunk_ps[c]

                            prev = None
                            for (c, j) in groups:
                                ensure_chunk(c)
                                wt_new = scores(c, j)
                                if prev is not None:
                                    pc, pj, pwt = prev
                                    po, prs = chunk_ps[pc]
                                    o_accum(pc, pj, pwt, po, prs)
                                    if pj == 2 * pc + 1:
                                        epilogue(pc, po, prs)
                                prev = (c, j, wt_new)
                            pc, pj, pwt = prev
                            po, prs = chunk_ps[pc]
                            o_accum(pc, pj, pwt, po, prs)
                            epilogue(pc, po, prs)

    nc.compile()
    return nc


_NC_CACHE = None


def _get_program():
    global _NC_CACHE
    if _NC_CACHE is None:
        _NC_CACHE = build_program()
    return _NC_CACHE


def make_in_maps(x, Wk, Wq, Wv):
    bf16 = ml_dtypes.bfloat16
    x = np.asarray(x, np.float32)
    xT = np.transpose(x, (0, 2, 1))                        # [B, E, T]
    # layout [t4][p][ek][512]: xT[e, t] with e = ek*128 + p
    xA = np.ascontiguousarray(
        xT.reshape(B, N_EB, 128, 4, 512).transpose(0, 3, 2, 1, 4)
    ).astype(bf16)

    def prep_w(W):   # [H,E] -> W.T [E,H] -> [hm][p][ek][128]
        WT = np.asarray(W, np.float32).T
        return np.ascontiguousarray(
            WT.reshape(N_EB, 128, N_HB, 128).transpose(2, 1, 0, 3)
        ).astype(bf16)

    WqT = prep_w(Wq)
    WkT = prep_w(Wk)
    WvT = np.ascontiguousarray(np.asarray(Wv, np.float32).T).astype(bf16)
    return [{"xA": xA[b], "WqT": WqT, "WkT": WkT, "WvT": WvT}
            for b in range(B)]


def kernel(x, Wk, Wq, Wv, _trace=False, _tmpdir=None):
    nc = _get_program()
    in_maps = make_in_maps(x, Wk, Wq, Wv)
    res = run_bass_kernel_spmd(nc, in_maps, list(range(N_CORES)),
                               trace=_trace, tmpdir=_tmpdir)
    out = np.stack([np.asarray(res.results[b]["out"]) for b in range(B)])
    out = out.astype(np.float32)
    if _trace:
        kernel.last_result = res
    return out
